# revision 52
# baseline (speedup 1.0000x reference)
"""Sliding-window attention (window=256) on 8 TRN2 NeuronCores.

Layout/algorithm notes
----------------------
Shapes: q,k,v [4,16,4096,64] fp32; B*H=64 (b,h) pairs sharded 8 per core
(fully local along sequence, no communication).

Per (b,h) and per 512-query block t (8 blocks per head):
  keys needed: [512t-256, 512t+512) = 6 key-chunks of 128 (global chunk
  index g = 4t-2+c, c=0..5; chunks with g<0 are skipped).
  S^T chunk  = matmul(lhsT=K^T[:,128g:128g+128] (fp32r [64,128]),
                      rhs=Q^T[:, 512t+qw_c]     (fp32r [64,|qw_c|]))
               -> PSUM [128,|qw_c|]  (scores transposed: [key, query]);
  qw_c is the chunk's valid query subrange (width 128..384); chunk pairs
  {c0,c2},{c1,c4},{c3,c5} share one PSUM bank each.
  P^T chunk  = exp(S^T * D^-1/2) via ACT (PSUM->SBUF, output rounded to
  fp32r), then out-of-band entries are zeroed by a {1,0} band-mask
  multiply split across DVE and GpSimd.  P^T slots are padded/zeroed to
  256-aligned windows so PV matmuls can share identical APs.
  O^T += matmul(lhsT=[V|1]-chunk (fp32r [128,65]), rhs=P^T slice) into
  one PSUM bank per 256-query column group; row 64 accumulates the
  softmax denominator (ones-column trick).
  Epilogue: copy O^T groups to SBUF, 4 PE transposes -> [128,65],
  reciprocal of the denominator column, per-partition scalar multiply,
  one DMA per block.

The emission order is software-pipelined (QK/exp/mask of block t is
emitted before PV/epilogue of block t-1) so the tensor engine always has
independent matmuls in flight while ACT/DVE/GpSimd work on the current
block.  Q^T/K^T are produced on-chip by PE transposes (fp32r, 4 per
PSUM bank + one wide copy).  fp32r matmuls measure ~1.7e-4 relative
error (TF32-like); the ACT exp table adds ~9e-6.
"""

import numpy as np

import concourse.bass as bass
import concourse.mybir as mybir
from concourse import bacc
from concourse.tile import TileContext
from concourse import bass_utils
from concourse.masks import make_identity

dt = mybir.dt

B, H, S, D = 4, 16, 4096, 64
W = 256                      # sliding window
N_CORES = 8
BH = (B * H) // N_CORES      # (b,h) pairs per core = 8
QT = 512                     # queries per block
NB = S // QT                 # blocks per (b,h) = 8
NT = S // 128                # 128-tiles per (b,h) = 32
SCALE = float(D) ** -0.5

# per-chunk query windows (relative to block start), c = 0..5
QW = [(max(0, 128 * (c - 2)), min(QT, 128 * (c - 2) + 384)) for c in range(6)]
# P^T slot windows, padded to 256-aligned PV column groups; the pad region
# (slot minus QW) is zero-filled once so PV matmuls share identical group APs
SLOT = [(0, 256), (0, 256), (0, 512), (0, 512), (256, 512), (256, 512)]
# PV column groups (2 per block) and their member chunks
PV_GROUPS = [(0, [0, 1, 2, 3]), (256, [2, 3, 4, 5])]
SLOT_BASE = {}
_off = 0
for _c in range(6):
    SLOT_BASE[_c] = _off - SLOT[_c][0]
    _off += SLOT[_c][1] - SLOT[_c][0]
PT_W = _off
# pad regions (cols) that must stay zero: slot minus computed window
PT_PADS = []
for _c in range(6):
    for _p0, _p1 in [(SLOT[_c][0], QW[_c][0]), (QW[_c][1], SLOT[_c][1])]:
        if _p1 > _p0:
            PT_PADS.append((SLOT_BASE[_c] + _p0, SLOT_BASE[_c] + _p1))

# S^T chunk -> (PSUM bank tag, column offset).  The expensive N=384 chunks
# (c2, c3) get private banks so their QK matmuls never serialize behind the
# bank partner's exp; the cheap chunks share the other two banks.
ST_BANK = {2: (0, 0), 3: (1, 0), 0: (2, 0), 5: (2, 128), 1: (3, 0), 4: (3, 256)}


def _band_valid_np():
    kl = np.arange(128)[:, None]
    m = np.arange(384)[None, :]
    return (m - 256 <= kl) & (kl <= m)


def _band_mask_np():
    """band[kl, m]: multiplicative mask, 1 where valid (m-256 <= kl <= m)."""
    return np.where(_band_valid_np(), np.float32(1.0), np.float32(0.0))


def _chunk_mask_ops(c):
    """For chunk c, list of (local j0, mega-mask col offset) for 128-wide
    subranges of qw_c that are not entirely valid."""
    j0s = []
    q0, q1 = QW[c]
    off = 128 * max(0, 2 - c)
    valid = _band_valid_np()
    for j0 in range(0, q1 - q0, 128):
        m0 = j0 + off
        if not valid[:, m0:m0 + 128].all():
            j0s.append((j0, m0))
    return j0s


MASK_OPS = {c: _chunk_mask_ops(c) for c in range(6)}


def build_core_kernel(n_bh=BH):
    nc = bacc.Bacc("TRN2", target_bir_lowering=False)
    qd = nc.dram_tensor("q", [n_bh * S, D], dt.float32, kind="ExternalInput")
    kd = nc.dram_tensor("k", [n_bh * S, D], dt.float32, kind="ExternalInput")
    vd = nc.dram_tensor("v", [n_bh * S, D], dt.float32, kind="ExternalInput")
    md = nc.dram_tensor("band_mask", [128, 384], dt.float32, kind="ExternalInput")
    od = nc.dram_tensor("o", [n_bh * S, D], dt.float32, kind="ExternalOutput")

    with TileContext(nc) as tc:
        with (
            tc.tile_pool(name="const", bufs=1) as constp,
            tc.tile_pool(name="bigio", bufs=2) as bigio,
            tc.tile_pool(name="work", bufs=2) as work,
            tc.tile_pool(name="pst", bufs=2, space="PSUM") as pst,     # prep transposes
            tc.tile_pool(name="psst", bufs=1, space="PSUM") as psst,   # S^T banks (4 tags)
            tc.tile_pool(name="psot", bufs=1, space="PSUM") as psot,   # O^T accum + epi transposes
        ):
            ident = constp.tile([128, 128], dt.float32)
            make_identity(nc, ident)
            ident_r = constp.tile([128, 128], dt.float32r)
            nc.vector.tensor_copy(ident_r[:], ident[:])
            mega = constp.tile([128, 384], dt.float32r)
            nc.gpsimd.dma_start(mega[:], md[:])   # cast {1,0} fp32 -> fp32r
            ones32 = constp.tile([128, NT], dt.float32)
            nc.vector.memset(ones32[:], 1.0)
            zeros32 = constp.tile([128, 128], dt.float32)
            nc.vector.memset(zeros32[:], 0.0)

            # persistent, manually double-buffered P^T tiles; pad columns are
            # zeroed once here and never written again
            pts = [constp.tile([128, PT_W], dt.float32r, name=f"ptbuf{i}")
                   for i in range(3)]
            for ptb in pts:
                for p0, p1 in PT_PADS:
                    for x0 in range(p0, p1, 128):
                        x1 = min(x0 + 128, p1)
                        nc.vector.tensor_copy(ptb[:, x0:x1], zeros32[:, 0:x1 - x0])

            for bh in range(n_bh):
                base = bh * S
                # ---- load natural-layout q/k/v, SWDGE-cast to fp32r
                qnat = bigio.tile([128, NT * D], dt.float32r, tag="qnat", name="qnat")
                knat = bigio.tile([128, NT * D], dt.float32r, tag="knat", name="knat")
                qsl = qd[base:base + S, :].rearrange("(t p) d -> p t d", p=128)
                ksl = kd[base:base + S, :].rearrange("(t p) d -> p t d", p=128)
                nc.gpsimd.dma_start(qnat[:].rearrange("p (t d) -> p t d", d=D), qsl)
                nc.gpsimd.dma_start(knat[:].rearrange("p (t d) -> p t d", d=D), ksl)

                vt = bigio.tile([128, NT * (D + 1)], dt.float32r, tag="vt", name="vt")
                vt3 = vt[:].rearrange("p (g e) -> p g e", e=D + 1)
                vsl = vd[base:base + S, :].rearrange("(g p) d -> p g d", p=128)
                nc.gpsimd.dma_start(vt3[:, :, 0:D], vsl)     # SWDGE casts fp32->fp32r
                # ones column: cast fp32 1.0 -> proper fp32r bits via DVE copy
                nc.vector.tensor_copy(vt3[:, :, D], ones32[:])

                # ---- transpose to Q^T/K^T [64, S] fp32r
                # 4 PE transposes into one PSUM bank, then a single wide copy
                qt = bigio.tile([64, S], dt.float32r, tag="qt", name="qt")
                kt = bigio.tile([64, S], dt.float32r, tag="kt", name="kt")
                for half, (nat, tr) in enumerate([(qnat, qt), (knat, kt)]):
                    for i0 in range(0, NT, 4):
                        ptr = pst.tile([64, 512], dt.float32r, tag="trp", name="ptr")
                        for u in range(4):
                            i = i0 + u
                            nc.tensor.transpose(
                                ptr[:, 128 * u:128 * (u + 1)],
                                nat[:, D * i:D * (i + 1)], ident_r[:])
                        if (i0 // 4 + half) % 2 == 0:
                            nc.scalar.copy(tr[:, 128 * i0:128 * (i0 + 4)], ptr[:])
                        else:
                            nc.vector.tensor_copy(
                                tr[:, 128 * i0:128 * (i0 + 4)], ptr[:])

                # ---- blocks
                def emit_qk(t):
                    """QK chunk matmuls + exp + masks for block t."""
                    chunks = [c for c in range(6) if 4 * t - 2 + c >= 0]
                    pt = pts[(bh * NB + t) % 3]
                    stt = {}
                    for c in chunks:
                        g = 4 * t - 2 + c
                        q0, q1 = QW[c]
                        w = q1 - q0
                        bank, boff = ST_BANK[c]
                        if bank not in stt:
                            stt[bank] = psst.tile(
                                [128, 512], dt.float32, tag=f"st{bank}",
                                name=f"st{bank}")
                        st = stt[bank]
                        nc.tensor.matmul(
                            st[:, boff:boff + w],
                            kt[:, 128 * g:128 * (g + 1)],
                            qt[:, QT * t + q0:QT * t + q1],
                            start=True, stop=True,
                        )
                        # exp(S^T * scale) -> P^T slot (fp32r), then zero the
                        # out-of-band entries with a {1,0} multiply (DVE/Pool)
                        po = SLOT_BASE[c] + q0
                        nc.scalar.activation(
                            pt[:, po:po + w], st[:, boff:boff + w],
                            mybir.ActivationFunctionType.Exp, scale=SCALE,
                        )
                        for mi_, (j0, m0) in enumerate(MASK_OPS[c]):
                            eng = nc.vector if (c + mi_) % 2 == 0 else nc.gpsimd
                            eng.tensor_tensor(
                                pt[:, po + j0:po + j0 + 128],
                                pt[:, po + j0:po + j0 + 128],
                                mega[:, m0:m0 + 128],
                                op=mybir.AluOpType.mult,
                            )

                def emit_pv(t):
                    """PV accumulation + normalize + transpose + store for t."""
                    chunks = [c for c in range(6) if 4 * t - 2 + c >= 0]
                    pt = pts[(bh * NB + t) % 3]
                    osb = work.tile([65, QT], dt.float32, tag="osb", name="osb")
                    for j, (col0, group) in enumerate(PV_GROUPS):
                        members = [c for c in group if c in chunks]
                        otj = psot.tile([65, 256], dt.float32, tag=f"ot{j}",
                                        name=f"ot{j}")
                        for mi, c in enumerate(members):
                            g = 4 * t - 2 + c
                            po = SLOT_BASE[c] + col0
                            nc.tensor.matmul(
                                otj[:], vt3[:, g, :], pt[:, po:po + 256],
                                start=(mi == 0), stop=(mi == len(members) - 1),
                            )
                        # DVE, not ACT: keeps the scalar engine free for exps
                        nc.vector.tensor_copy(osb[:, col0:col0 + 256], otj[:])

                    otr = psot.tile([128, 4 * 65], dt.float32, tag="ot0", name="otr")
                    for j in range(4):
                        nc.tensor.transpose(
                            otr[:, 65 * j:65 * (j + 1)],
                            osb[:, 128 * j:128 * (j + 1)], ident[0:65, 0:65],
                        )
                    rc = work.tile([128, 4], dt.float32, tag="rc", name="rc")
                    otr3 = otr[:].rearrange("p (j e) -> p j e", e=65)
                    nc.vector.reciprocal(rc[:], otr3[:, :, 64])
                    outsb = work.tile([128, 4 * D], dt.float32, tag="outsb",
                                      name="outsb")
                    for j in range(4):
                        nc.vector.tensor_scalar_mul(
                            outsb[:, D * j:D * (j + 1)],
                            otr[:, 65 * j:65 * j + 64],
                            rc[:, j:j + 1],
                        )
                    osl = od[base + QT * t:base + QT * (t + 1), :].rearrange(
                        "(j p) d -> p j d", p=128)
                    nc.sync.dma_start(
                        osl, outsb[:].rearrange("p (j d) -> p j d", d=D))

                # software pipeline: emit QK(t) ahead of PV(t-1) so the PE
                # always has independent matmul work while exp/masks of the
                # current block complete on ACT/DVE/Pool
                for t in range(NB):
                    emit_qk(t)
                    if t > 0:
                        emit_pv(t - 1)
                emit_pv(NB - 1)

    nc.finalize()
    return nc


_NC_CACHE = []


def _get_nc():
    if not _NC_CACHE:
        _NC_CACHE.append(build_core_kernel())
    return _NC_CACHE[0]


def make_in_maps(q, k, v):
    qr = np.ascontiguousarray(np.asarray(q, dtype=np.float32).reshape(B * H, S, D))
    kr = np.ascontiguousarray(np.asarray(k, dtype=np.float32).reshape(B * H, S, D))
    vr = np.ascontiguousarray(np.asarray(v, dtype=np.float32).reshape(B * H, S, D))
    band = np.ascontiguousarray(_band_mask_np())

    in_maps = []
    for i in range(N_CORES):
        in_maps.append({
            "q": np.ascontiguousarray(qr[BH * i:BH * (i + 1)].reshape(BH * S, D)),
            "k": np.ascontiguousarray(kr[BH * i:BH * (i + 1)].reshape(BH * S, D)),
            "v": np.ascontiguousarray(vr[BH * i:BH * (i + 1)].reshape(BH * S, D)),
            "band_mask": band,
        })
    return in_maps


def gather_out(res):
    out = np.empty((B * H, S, D), dtype=np.float32)
    for i in range(N_CORES):
        out[BH * i:BH * (i + 1)] = res.results[i]["o"].reshape(BH, S, D)
    return out.reshape(B, H, S, D)


def kernel(q, k, v):
    nc = _get_nc()
    in_maps = make_in_maps(q, k, v)
    res = bass_utils.run_bass_kernel_spmd(nc, in_maps, core_ids=list(range(N_CORES)))
    return gather_out(res)


# revision 54
# speedup vs baseline: 1.0062x; 1.0062x over previous
"""Sliding-window attention (window=256) on 8 TRN2 NeuronCores.

Layout/algorithm notes
----------------------
Shapes: q,k,v [4,16,4096,64] fp32; B*H=64 (b,h) pairs sharded 8 per core
(fully local along sequence, no communication).

Per (b,h) and per 512-query block t (8 blocks per head):
  keys needed: [512t-256, 512t+512) = 6 key-chunks of 128 (global chunk
  index g = 4t-2+c, c=0..5; chunks with g<0 are skipped).
  S^T chunk  = matmul(lhsT=K^T[:,128g:128g+128] (fp32r [64,128]),
                      rhs=Q^T[:, 512t+qw_c]     (fp32r [64,|qw_c|]))
               -> PSUM [128,|qw_c|]  (scores transposed: [key, query]);
  qw_c is the chunk's valid query subrange (width 128..384); chunk pairs
  {c0,c2},{c1,c4},{c3,c5} share one PSUM bank each.
  P^T chunk  = exp(S^T * D^-1/2) via ACT (PSUM->SBUF, output rounded to
  fp32r), then out-of-band entries are zeroed by a {1,0} band-mask
  multiply split across DVE and GpSimd.  P^T slots are padded/zeroed to
  256-aligned windows so PV matmuls can share identical APs.
  O^T += matmul(lhsT=[V|1]-chunk (fp32r [128,65]), rhs=P^T slice) into
  one PSUM bank per 256-query column group; row 64 accumulates the
  softmax denominator (ones-column trick).
  Epilogue: copy O^T groups to SBUF, 4 PE transposes -> [128,65],
  reciprocal of the denominator column, per-partition scalar multiply,
  one DMA per block.

The emission order is software-pipelined (QK/exp/mask of block t is
emitted before PV/epilogue of block t-1) so the tensor engine always has
independent matmuls in flight while ACT/DVE/GpSimd work on the current
block.  Q^T/K^T are produced on-chip by PE transposes (fp32r, 4 per
PSUM bank + one wide copy).  fp32r matmuls measure ~1.7e-4 relative
error (TF32-like); the ACT exp table adds ~9e-6.
"""

import numpy as np

import concourse.bass as bass
import concourse.mybir as mybir
from concourse import bacc
from concourse.tile import TileContext
from concourse import bass_utils
from concourse.masks import make_identity

dt = mybir.dt

B, H, S, D = 4, 16, 4096, 64
W = 256                      # sliding window
N_CORES = 8
BH = (B * H) // N_CORES      # (b,h) pairs per core = 8
QT = 512                     # queries per block
NB = S // QT                 # blocks per (b,h) = 8
NT = S // 128                # 128-tiles per (b,h) = 32
SCALE = float(D) ** -0.5

# per-chunk query windows (relative to block start), c = 0..5
QW = [(max(0, 128 * (c - 2)), min(QT, 128 * (c - 2) + 384)) for c in range(6)]
# P^T slot windows, padded to 256-aligned PV column groups; the pad region
# (slot minus QW) is zero-filled once so PV matmuls share identical group APs
SLOT = [(0, 256), (0, 256), (0, 512), (0, 512), (256, 512), (256, 512)]
# PV column groups (2 per block) and their member chunks
PV_GROUPS = [(0, [0, 1, 2, 3]), (256, [2, 3, 4, 5])]
SLOT_BASE = {}
_off = 0
for _c in range(6):
    SLOT_BASE[_c] = _off - SLOT[_c][0]
    _off += SLOT[_c][1] - SLOT[_c][0]
PT_W = _off
# pad regions (cols) that must stay zero: slot minus computed window
PT_PADS = []
for _c in range(6):
    for _p0, _p1 in [(SLOT[_c][0], QW[_c][0]), (QW[_c][1], SLOT[_c][1])]:
        if _p1 > _p0:
            PT_PADS.append((SLOT_BASE[_c] + _p0, SLOT_BASE[_c] + _p1))

# S^T chunk pairs sharing one PSUM bank (widths sum <= 512 fp32)
ST_BANK = {0: (0, 0), 2: (0, 128), 1: (1, 0), 4: (1, 256), 3: (2, 0), 5: (2, 384)}


def _band_valid_np():
    kl = np.arange(128)[:, None]
    m = np.arange(384)[None, :]
    return (m - 256 <= kl) & (kl <= m)


def _band_mask_np():
    """band[kl, m]: multiplicative mask, 1 where valid (m-256 <= kl <= m)."""
    return np.where(_band_valid_np(), np.float32(1.0), np.float32(0.0))


def _chunk_mask_ops(c):
    """For chunk c, list of (local j0, mega-mask col offset) for 128-wide
    subranges of qw_c that are not entirely valid."""
    j0s = []
    q0, q1 = QW[c]
    off = 128 * max(0, 2 - c)
    valid = _band_valid_np()
    for j0 in range(0, q1 - q0, 128):
        m0 = j0 + off
        if not valid[:, m0:m0 + 128].all():
            j0s.append((j0, m0))
    return j0s


MASK_OPS = {c: _chunk_mask_ops(c) for c in range(6)}


def build_core_kernel(n_bh=BH):
    nc = bacc.Bacc("TRN2", target_bir_lowering=False)
    qd = nc.dram_tensor("q", [n_bh * S, D], dt.float32, kind="ExternalInput")
    kd = nc.dram_tensor("k", [n_bh * S, D], dt.float32, kind="ExternalInput")
    vd = nc.dram_tensor("v", [n_bh * S, D], dt.float32, kind="ExternalInput")
    md = nc.dram_tensor("band_mask", [128, 384], dt.float32, kind="ExternalInput")
    od = nc.dram_tensor("o", [n_bh * S, D], dt.float32, kind="ExternalOutput")

    with TileContext(nc) as tc:
        with (
            tc.tile_pool(name="const", bufs=1) as constp,
            tc.tile_pool(name="bigio", bufs=2) as bigio,
            tc.tile_pool(name="work", bufs=2) as work,
            tc.tile_pool(name="pst", bufs=3, space="PSUM") as pst,     # prep transposes
            tc.tile_pool(name="psst", bufs=1, space="PSUM") as psst,   # S^T pair-banks (3 tags)
            tc.tile_pool(name="psot", bufs=1, space="PSUM") as psot,   # O^T accum + epi transposes
        ):
            ident = constp.tile([128, 128], dt.float32)
            make_identity(nc, ident)
            ident_r = constp.tile([128, 128], dt.float32r)
            nc.vector.tensor_copy(ident_r[:], ident[:])
            mega = constp.tile([128, 384], dt.float32r)
            nc.gpsimd.dma_start(mega[:], md[:])   # cast {1,0} fp32 -> fp32r
            ones32 = constp.tile([128, NT], dt.float32)
            nc.vector.memset(ones32[:], 1.0)
            zeros32 = constp.tile([128, 128], dt.float32)
            nc.vector.memset(zeros32[:], 0.0)

            # persistent, manually double-buffered P^T tiles; pad columns are
            # zeroed once here and never written again
            pts = [constp.tile([128, PT_W], dt.float32r, name=f"ptbuf{i}")
                   for i in range(3)]
            for ptb in pts:
                for p0, p1 in PT_PADS:
                    for x0 in range(p0, p1, 128):
                        x1 = min(x0 + 128, p1)
                        nc.vector.tensor_copy(ptb[:, x0:x1], zeros32[:, 0:x1 - x0])

            for bh in range(n_bh):
                base = bh * S
                # ---- load natural-layout q/k/v, SWDGE-cast to fp32r
                qnat = bigio.tile([128, NT * D], dt.float32r, tag="qnat", name="qnat")
                knat = bigio.tile([128, NT * D], dt.float32r, tag="knat", name="knat")
                qsl = qd[base:base + S, :].rearrange("(t p) d -> p t d", p=128)
                ksl = kd[base:base + S, :].rearrange("(t p) d -> p t d", p=128)
                nc.gpsimd.dma_start(qnat[:].rearrange("p (t d) -> p t d", d=D), qsl)
                nc.gpsimd.dma_start(knat[:].rearrange("p (t d) -> p t d", d=D), ksl)

                vt = bigio.tile([128, NT * (D + 1)], dt.float32r, tag="vt", name="vt")
                vt3 = vt[:].rearrange("p (g e) -> p g e", e=D + 1)
                vsl = vd[base:base + S, :].rearrange("(g p) d -> p g d", p=128)
                nc.gpsimd.dma_start(vt3[:, :, 0:D], vsl)     # SWDGE casts fp32->fp32r
                # ones column: cast fp32 1.0 -> proper fp32r bits via DVE copy
                nc.vector.tensor_copy(vt3[:, :, D], ones32[:])

                # ---- transpose to Q^T/K^T [64, S] fp32r
                # 4 PE transposes into one PSUM bank, then a single wide copy
                qt = bigio.tile([64, S], dt.float32r, tag="qt", name="qt")
                kt = bigio.tile([64, S], dt.float32r, tag="kt", name="kt")
                for half, (nat, tr) in enumerate([(qnat, qt), (knat, kt)]):
                    for i0 in range(0, NT, 4):
                        ptr = pst.tile([64, 512], dt.float32r, tag="trp", name="ptr")
                        for u in range(4):
                            i = i0 + u
                            nc.tensor.transpose(
                                ptr[:, 128 * u:128 * (u + 1)],
                                nat[:, D * i:D * (i + 1)], ident_r[:])
                        if (i0 // 4 + half) % 2 == 0:
                            nc.scalar.copy(tr[:, 128 * i0:128 * (i0 + 4)], ptr[:])
                        else:
                            nc.vector.tensor_copy(
                                tr[:, 128 * i0:128 * (i0 + 4)], ptr[:])

                # ---- blocks
                def emit_qk(t):
                    """QK chunk matmuls + exp + masks for block t."""
                    chunks = [c for c in range(6) if 4 * t - 2 + c >= 0]
                    pt = pts[(bh * NB + t) % 3]
                    stt = {}
                    for c in chunks:
                        g = 4 * t - 2 + c
                        q0, q1 = QW[c]
                        w = q1 - q0
                        bank, boff = ST_BANK[c]
                        if bank not in stt:
                            stt[bank] = psst.tile(
                                [128, 512], dt.float32, tag=f"st{bank}",
                                name=f"st{bank}")
                        st = stt[bank]
                        nc.tensor.matmul(
                            st[:, boff:boff + w],
                            kt[:, 128 * g:128 * (g + 1)],
                            qt[:, QT * t + q0:QT * t + q1],
                            start=True, stop=True,
                        )
                        # exp(S^T * scale) -> P^T slot (fp32r), then zero the
                        # out-of-band entries with a {1,0} multiply (DVE/Pool)
                        po = SLOT_BASE[c] + q0
                        nc.scalar.activation(
                            pt[:, po:po + w], st[:, boff:boff + w],
                            mybir.ActivationFunctionType.Exp, scale=SCALE,
                        )
                        for mi_, (j0, m0) in enumerate(MASK_OPS[c]):
                            eng = nc.vector if (c + mi_) % 2 == 0 else nc.gpsimd
                            eng.tensor_tensor(
                                pt[:, po + j0:po + j0 + 128],
                                pt[:, po + j0:po + j0 + 128],
                                mega[:, m0:m0 + 128],
                                op=mybir.AluOpType.mult,
                            )

                def emit_pv(t):
                    """PV accumulation + normalize + transpose + store for t."""
                    chunks = [c for c in range(6) if 4 * t - 2 + c >= 0]
                    pt = pts[(bh * NB + t) % 3]
                    osb = work.tile([65, QT], dt.float32, tag="osb", name="osb")
                    for j, (col0, group) in enumerate(PV_GROUPS):
                        members = [c for c in group if c in chunks]
                        otj = psot.tile([65, 256], dt.float32, tag=f"ot{j}",
                                        name=f"ot{j}")
                        for mi, c in enumerate(members):
                            g = 4 * t - 2 + c
                            po = SLOT_BASE[c] + col0
                            nc.tensor.matmul(
                                otj[:], vt3[:, g, :], pt[:, po:po + 256],
                                start=(mi == 0), stop=(mi == len(members) - 1),
                            )
                        # DVE, not ACT: keeps the scalar engine free for exps
                        nc.vector.tensor_copy(osb[:, col0:col0 + 256], otj[:])

                    otr = psot.tile([128, 4 * 65], dt.float32, tag="ot0", name="otr")
                    for j in range(4):
                        nc.tensor.transpose(
                            otr[:, 65 * j:65 * (j + 1)],
                            osb[:, 128 * j:128 * (j + 1)], ident[0:65, 0:65],
                        )
                    rc = work.tile([128, 4], dt.float32, tag="rc", name="rc")
                    otr3 = otr[:].rearrange("p (j e) -> p j e", e=65)
                    nc.vector.reciprocal(rc[:], otr3[:, :, 64])
                    outsb = work.tile([128, 4 * D], dt.float32, tag="outsb",
                                      name="outsb")
                    for j in range(4):
                        nc.vector.tensor_scalar_mul(
                            outsb[:, D * j:D * (j + 1)],
                            otr[:, 65 * j:65 * j + 64],
                            rc[:, j:j + 1],
                        )
                    osl = od[base + QT * t:base + QT * (t + 1), :].rearrange(
                        "(j p) d -> p j d", p=128)
                    nc.sync.dma_start(
                        osl, outsb[:].rearrange("p (j d) -> p j d", d=D))

                # software pipeline: emit QK(t) ahead of PV(t-1) so the PE
                # always has independent matmul work while exp/masks of the
                # current block complete on ACT/DVE/Pool
                for t in range(NB):
                    emit_qk(t)
                    if t > 0:
                        emit_pv(t - 1)
                emit_pv(NB - 1)

    nc.finalize()
    return nc


_NC_CACHE = []


def _get_nc():
    if not _NC_CACHE:
        _NC_CACHE.append(build_core_kernel())
    return _NC_CACHE[0]


def make_in_maps(q, k, v):
    qr = np.ascontiguousarray(np.asarray(q, dtype=np.float32).reshape(B * H, S, D))
    kr = np.ascontiguousarray(np.asarray(k, dtype=np.float32).reshape(B * H, S, D))
    vr = np.ascontiguousarray(np.asarray(v, dtype=np.float32).reshape(B * H, S, D))
    band = np.ascontiguousarray(_band_mask_np())

    in_maps = []
    for i in range(N_CORES):
        in_maps.append({
            "q": np.ascontiguousarray(qr[BH * i:BH * (i + 1)].reshape(BH * S, D)),
            "k": np.ascontiguousarray(kr[BH * i:BH * (i + 1)].reshape(BH * S, D)),
            "v": np.ascontiguousarray(vr[BH * i:BH * (i + 1)].reshape(BH * S, D)),
            "band_mask": band,
        })
    return in_maps


def gather_out(res):
    out = np.empty((B * H, S, D), dtype=np.float32)
    for i in range(N_CORES):
        out[BH * i:BH * (i + 1)] = res.results[i]["o"].reshape(BH, S, D)
    return out.reshape(B, H, S, D)


def kernel(q, k, v):
    nc = _get_nc()
    in_maps = make_in_maps(q, k, v)
    res = bass_utils.run_bass_kernel_spmd(nc, in_maps, core_ids=list(range(N_CORES)))
    return gather_out(res)
